# revision 5
# baseline (speedup 1.0000x reference)
"""GAT edge-softmax (segment softmax over 400K segments) on 8 Trainium2
NeuronCores, written in raw Bass — fp16 streaming version.

Structure
---------
L1 (device, DMA-bound): the 3.2M edges are sharded contiguously across
the 8 cores; with 8 heads and E edges/head, core c gets exactly head
c's edges, so the attention vector w = a_l * a_r is a per-core
constant. The host folds w and the f32->f16 conversion into one pass
(xiw = x_i * w, xj both fp16), halving HBM traffic vs f32 — a single
SP-queue DMA stream measures 283us/core for the 102.4 MB, and this
kernel reaches 292us. Compute runs in 2-chunk "super" units: one
in-place fp16 multiply (DVE 2x mode), then a halving tree for the
64-wide window sum — first step out-of-place into a small pyramid
buffer (frees the input slot early for prefetch), remaining steps in
place, all fp16 2x — then ACT Exp writes fp16 z. The per-sweep z
write-back is issued from the ACT queue, where it orders naturally
after the last Exp instead of stalling the SP DMA stream.

Host (pure index shuffling): z is bucketed by destination segment into
a dense zero-padded pad-major [pad, segments] fp16 layout,
pre-partitioned so each segment lives on exactly one core — the
cross-device segment reduction of the hint becomes unnecessary, and
the empty padding slots are exact zeros under sum.

L2 (device, small): whole-buffer single DMAs (in on SP, out on ACT
queue); DVE sums the pad axis with a halving tree (fp16 2x), adds
1e-16, reciprocal, and one 2x broadcast multiply normalizes in place.

Host: alphas are gathered back to the original edge order (f32 out).

The reference's max-subtraction is skipped: e = sum_d xi*xj*w has
sigma ~0.12 (w is glorot-initialized), so |e| < ~1 over 3.2M samples;
exp cannot overflow fp16 and alpha = z/(sum z + 1e-16) differs from
the max-subtracted form by <=2e-16 relative.

Accuracy budget: fp16 inputs + fp16 tree rounding give max rel err
~2e-3 on alpha, vs the 2e-2 gate.

Platform constraints honored (found the hard way):
- walrus permits at most ONE semaphore wait attached per instruction ->
  standalone wait instructions, no TileContext.
- dependent same-engine ops still need semaphore sync (engine frees
  before writes drain); the race detector enforces this.
- multi-queue BULK DMA is ~1.6x WORSE on real HW than a single queue
  (CoreSim models it as 2x better — do not trust it there); only the
  small per-sweep write-backs go on the ACT queue.
"""
import contextlib
import sys

sys.path.insert(0, "/opt/trn_rl_repo")

import numpy as np

import concourse.bass as bass
from concourse import mybir
from concourse.bass_utils import run_bass_kernel_spmd

F16 = mybir.dt.float16
F32 = mybir.dt.float32
P = 128
D = 64
NCORES = 8
RPP = 125  # edge rows per partition per L1 chunk

_cache = {}


def _build_l1(epc, rpp=RPP, repeat=1):
    """Per-core score kernel: z[p, c*rpp+r] = exp(sum_d xiw*xj) of edge
    c*(128*rpp) + p*rpp + r. Inputs xiw/xj [epc, 64] f16; z [128, epc/128]
    f16. Compute in 2-chunk super units; 25 chunks/sweep = 12 supers +
    tail chunk (dedicated slot 4; super chunks cycle slots 0-3)."""
    chunk_edges = P * rpp
    assert epc % chunk_edges == 0
    nchunks_data = epc // chunk_edges
    assert nchunks_data % 2 == 1
    nsup = nchunks_data // 2
    free = rpp * D
    srpp = 2 * rpp
    zcols = epc // P
    Exp = mybir.ActivationFunctionType.Exp

    nc = bass.Bass()
    xiw = nc.declare_dram_parameter("xiw", [epc, D], F16, isOutput=False)
    xj = nc.declare_dram_parameter("xj", [epc, D], F16, isOutput=False)
    z_out = nc.declare_dram_parameter("z", [P, zcols], F16, isOutput=True)

    xi_t = xiw[:].rearrange("(c p r) d -> c p (r d)", p=P, r=rpp)
    xj_t = xj[:].rearrange("(c p r) d -> c p (r d)", p=P, r=rpp)

    UPS = nsup + 1  # units per sweep: supers then the tail chunk
    nunits = UPS * repeat
    nchunks = nchunks_data * repeat

    def chunk_slot(c):
        dc = c % nchunks_data
        return 4 if dc == nchunks_data - 1 else dc % 4

    def chunk_unit(c):
        sweep, dc = divmod(c, nchunks_data)
        return sweep * UPS + min(dc // 2, nsup)

    def unit_chunks(g):
        sweep, u = divmod(g, UPS)
        base = sweep * nchunks_data
        if u < nsup:
            return [base + 2 * u, base + 2 * u + 1]
        return [base + 2 * nsup]

    slot_uses = {}
    use_idx = {}
    for c in range(nchunks):
        b = chunk_slot(c)
        slot_uses[b] = slot_uses.get(b, 0) + 1
        use_idx[c] = slot_uses[b]

    # DVE ops per unit: mult, t1 (out-of-place), t2..t5 (in-place), t6
    order = [(g, k) for g in range(nunits) for k in range(7)]
    val = {}
    n = 0
    for g, k in order:
        n += 1
        val[(g, k)] = n

    st = contextlib.ExitStack()
    with st:
        ti = st.enter_context(nc.sbuf_tensor("ti", [P, 5 * free], F16))
        tj = st.enter_context(nc.sbuf_tensor("tj", [P, 5 * free], F16))
        u1 = [st.enter_context(nc.sbuf_tensor(f"u1{k}", [P, srpp * 32], F16)) for k in range(2)]
        er = [st.enter_context(nc.sbuf_tensor(f"er{k}", [P, srpp], F16)) for k in range(2)]
        zbuf = st.enter_context(nc.sbuf_tensor("zbuf", [P, zcols], F16))
        smi = [st.enter_context(nc.semaphore(f"smi{k}")) for k in range(5)]
        smj = [st.enter_context(nc.semaphore(f"smj{k}")) for k in range(5)]
        dve_sem = st.enter_context(nc.semaphore("dve_sem"))
        act_sem = st.enter_context(nc.semaphore("act_sem"))
        out_sem = st.enter_context(nc.semaphore("out_sem"))
        block = st.enter_context(nc.Block())

        @block.sync
        def _(sync):
            prev_use = {}
            for c in range(nchunks):
                b = chunk_slot(c)
                if b in prev_use:
                    # slot reuse: the unit that consumed the previous
                    # occupant must have finished t1 (frees ti+tj)
                    sync.wait_ge(dve_sem, val[(chunk_unit(prev_use[b]), 1)])
                prev_use[b] = c
                dc = c % nchunks_data
                sync.dma_start(
                    out=ti[:, b * free : (b + 1) * free], in_=xi_t[dc]
                ).then_inc(smi[b], 16)
                sync.dma_start(
                    out=tj[:, b * free : (b + 1) * free], in_=xj_t[dc]
                ).then_inc(smj[b], 16)
            sync.wait_ge(out_sem, 16 * repeat)

        @block.vector
        def _(vector):
            for g, k in order:
                chunks = unit_chunks(g)
                b0 = chunk_slot(chunks[0])
                width = srpp if len(chunks) == 2 else rpp
                tiv = ti[:, b0 * free : b0 * free + width * D]
                tjv = tj[:, b0 * free : b0 * free + width * D]
                ub = u1[g % 2]
                eb = er[g % 2]
                uv = ub[:, : width * 32].rearrange("p (r w) -> p r w", w=32)
                if k == 0:
                    for c in chunks:
                        vector.wait_ge(smi[chunk_slot(c)], 16 * use_idx[c])
                        vector.wait_ge(smj[chunk_slot(c)], 16 * use_idx[c])
                    nc.vector.tensor_tensor(
                        out=tiv, in0=tiv, in1=tjv, op=mybir.AluOpType.mult
                    ).then_inc(dve_sem, 1)
                elif k == 1:
                    if g >= 2:
                        # u1[g%2] reuse: unit g-2's t6 must have read it
                        vector.wait_ge(dve_sem, val[(g - 2, 6)])
                    vector.wait_ge(dve_sem, val[(g, 0)])
                    tv = tiv.rearrange("p (r d) -> p r d", d=D)
                    nc.vector.tensor_tensor(
                        out=uv, in0=tv[:, :, 0:32], in1=tv[:, :, 32:64],
                        op=mybir.AluOpType.add,
                    ).then_inc(dve_sem, 1)
                elif k < 6:
                    w = 32 >> (k - 1)  # 16, 8, 4, 2
                    vector.wait_ge(dve_sem, val[(g, k - 1)])
                    nc.vector.tensor_tensor(
                        out=uv[:, :, 0:w], in0=uv[:, :, 0:w],
                        in1=uv[:, :, w : 2 * w], op=mybir.AluOpType.add,
                    ).then_inc(dve_sem, 1)
                else:
                    if g >= 2:
                        # er[g%2] reuse: ACT of unit g-2 must have read it
                        vector.wait_ge(act_sem, g - 1)
                    vector.wait_ge(dve_sem, val[(g, 5)])
                    nc.vector.tensor_tensor(
                        out=eb[:, :width].rearrange("p (r o) -> p r o", o=1),
                        in0=uv[:, :, 0:1], in1=uv[:, :, 1:2],
                        op=mybir.AluOpType.add,
                    ).then_inc(dve_sem, 1)

        @block.scalar
        def _(scalar):
            for g in range(nunits):
                sweep, u = divmod(g, UPS)
                chunks = unit_chunks(g)
                width = srpp if len(chunks) == 2 else rpp
                col0 = (chunks[0] % nchunks_data) * rpp
                if u == 0 and sweep >= 1:
                    # zbuf overwrite must not race the async z_out read
                    scalar.wait_ge(out_sem, 16 * sweep)
                scalar.wait_ge(dve_sem, val[(g, 6)])
                nc.scalar.activation(
                    out=zbuf[:, col0 : col0 + width],
                    in_=er[g % 2][:, :width],
                    func=Exp,
                ).then_inc(act_sem, 1)
                if u == UPS - 1:
                    # sweep's last exp drained -> write z back; in-order
                    # ACT queue also orders this before next sweep's exps
                    scalar.wait_ge(act_sem, UPS * (sweep + 1))
                    if sweep >= 1:
                        scalar.wait_ge(out_sem, 16 * sweep)
                    nc.scalar.dma_start(out=z_out[:], in_=zbuf[:]).then_inc(
                        out_sem, 16
                    )

    return nc


def _tree_steps(pad):
    steps = []
    q = pad
    while q > 2:
        h = q // 2
        steps.append((h, q))
        q = q - h
    return steps


def _build_l2(ntA, padA, ntB, padB, repeat=1):
    """Per-core segment normalize, two count-classes, pad-major fp16:
    ap[p,q,t] = zp[p,q,t] / (sum_q zp[p,q,t] + 1e-16) for each class.
    Class B (ntB=0 disallowed; pass ntB>=1 zero-filled when empty)."""
    assert padA % 2 == 0 and padA >= 4 and padB % 2 == 0 and padB >= 4
    nc = bass.Bass()
    zpA = nc.declare_dram_parameter("zpA", [P, padA, ntA], F16, isOutput=False)
    zpB = nc.declare_dram_parameter("zpB", [P, padB, ntB], F16, isOutput=False)
    apA = nc.declare_dram_parameter("apA", [P, padA, ntA], F16, isOutput=True)
    apB = nc.declare_dram_parameter("apB", [P, padB, ntB], F16, isOutput=True)

    phases = [
        dict(nt=ntA, pad=padA, steps=_tree_steps(padA)),
        dict(nt=ntB, pad=padB, steps=_tree_steps(padB)),
    ]
    for ph in phases:
        ph["dops"] = len(ph["steps"]) + 5
    DOPS = sum(ph["dops"] for ph in phases)
    w1_elems = max((ph["pad"] // 2) * ph["nt"] for ph in phases)
    s_elems = max(ph["nt"] for ph in phases)

    st = contextlib.ExitStack()
    with st:
        zbA = [st.enter_context(nc.sbuf_tensor(f"zbA{k}", [P, padA * ntA], F16)) for k in range(2)]
        zbB = [st.enter_context(nc.sbuf_tensor(f"zbB{k}", [P, padB * ntB], F16)) for k in range(2)]
        w1 = st.enter_context(nc.sbuf_tensor("w1", [P, w1_elems], F16))
        s = st.enter_context(nc.sbuf_tensor("s", [P, s_elems], F32))
        rec = st.enter_context(nc.sbuf_tensor("rec", [P, s_elems], F16))
        sminA = [st.enter_context(nc.semaphore(f"sminA{k}")) for k in range(2)]
        sminB = [st.enter_context(nc.semaphore(f"sminB{k}")) for k in range(2)]
        dve_sem = st.enter_context(nc.semaphore("dve_sem"))
        outA_sem = st.enter_context(nc.semaphore("outA_sem"))
        outB_sem = st.enter_context(nc.semaphore("outB_sem"))
        block = st.enter_context(nc.Block())

        phases[0].update(zb=zbA, smin=sminA, out_sem=outA_sem, zp=zpA, ap=apA)
        phases[1].update(zb=zbB, smin=sminB, out_sem=outB_sem, zp=zpB, ap=apB)

        @block.sync
        def _(sync):
            for sw in range(repeat):
                b = sw % 2
                for ph in phases:
                    if sw >= 2:
                        sync.wait_ge(ph["out_sem"], 16 * (sw - 1))
                    sync.dma_start(out=ph["zb"][b][:], in_=ph["zp"][:]).then_inc(
                        ph["smin"][b], 16
                    )
            for ph in phases:
                sync.wait_ge(ph["out_sem"], 16 * repeat)

        @block.vector
        def _(vector):
            for sw in range(repeat):
                b = sw % 2
                k = DOPS * sw  # running dve_sem value
                for pi, ph in enumerate(phases):
                    nt, pad = ph["nt"], ph["pad"]
                    vector.wait_ge(ph["smin"][b], 16 * (sw // 2 + 1))
                    if sw >= 1 and pi == 0:
                        # w1/s/rec write-after-read vs prev sweep's phase B
                        vector.wait_ge(dve_sem, DOPS * sw)
                    zv = ph["zb"][b][:].rearrange("p (q t) -> p q t", t=nt)
                    wv = w1[:, : (pad // 2) * nt].rearrange(
                        "p (q t) -> p q t", t=nt
                    )
                    first = True
                    for h, qq in ph["steps"]:
                        if first:
                            if pi == 1:
                                # w1 write-after-read vs phase A's final
                                vector.wait_ge(dve_sem, k)
                            nc.vector.tensor_tensor(
                                out=wv[:, 0:h, :], in0=zv[:, 0:h, :],
                                in1=zv[:, qq - h : qq, :],
                                op=mybir.AluOpType.add,
                            ).then_inc(dve_sem, 1)
                        else:
                            vector.wait_ge(dve_sem, k)
                            nc.vector.tensor_tensor(
                                out=wv[:, 0:h, :], in0=wv[:, 0:h, :],
                                in1=wv[:, qq - h : qq, :],
                                op=mybir.AluOpType.add,
                            ).then_inc(dve_sem, 1)
                        first = False
                        k += 1
                    vector.wait_ge(dve_sem, k)
                    nc.vector.tensor_tensor(
                        out=s[:, :nt].rearrange("p (o t) -> p o t", o=1),
                        in0=wv[:, 0:1, :], in1=wv[:, 1:2, :],
                        op=mybir.AluOpType.add,
                    ).then_inc(dve_sem, 1)
                    k += 1
                    vector.wait_ge(dve_sem, k)
                    nc.vector.tensor_scalar_add(
                        out=s[:, :nt], in0=s[:, :nt], scalar1=1e-16
                    ).then_inc(dve_sem, 1)
                    k += 1
                    vector.wait_ge(dve_sem, k)
                    nc.vector.reciprocal(out=s[:, :nt], in_=s[:, :nt]).then_inc(
                        dve_sem, 1
                    )
                    k += 1
                    vector.wait_ge(dve_sem, k)
                    # clamped f16 cast: empty segments have recip 1e16
                    # which would overflow f16; real segments are < 3
                    nc.vector.tensor_scalar(
                        out=rec[:, :nt], in0=s[:, :nt], scalar1=60000.0,
                        scalar2=None, op0=mybir.AluOpType.min,
                    ).then_inc(dve_sem, 1)
                    k += 1
                    vector.wait_ge(dve_sem, k)
                    rec_ap = rec[:, :nt]
                    rb = bass.AP(
                        tensor=rec_ap.tensor, offset=rec_ap.offset,
                        ap=[rec_ap.ap[0], [0, pad], rec_ap.ap[1]],
                    )
                    nc.vector.tensor_tensor(
                        out=zv, in0=zv, in1=rb, op=mybir.AluOpType.mult
                    ).then_inc(dve_sem, 1)
                    k += 1

        @block.scalar
        def _(scalar):
            for sw in range(repeat):
                b = sw % 2
                k = DOPS * sw
                for ph in phases:
                    k += ph["dops"]
                    scalar.wait_ge(dve_sem, k)
                    if sw >= 1:
                        scalar.wait_ge(ph["out_sem"], 16 * sw)
                    nc.scalar.dma_start(
                        out=ph["ap"][:], in_=ph["zb"][b][:]
                    ).then_inc(ph["out_sem"], 16)

    return nc


def _run_spmd(nc, in_maps, core_ids, tries=3):
    last = None
    for attempt in range(tries):
        try:
            return run_bass_kernel_spmd(nc, in_maps, core_ids)
        except Exception as e:  # axon/NRT execution is occasionally flaky
            last = e
    raise last


def _kernel_numpy(x_i, x_j, a, idx, num_nodes):
    """Host fallback for shapes the device path doesn't cover."""
    H = a.shape[0]
    Dd = a.shape[2] // 2
    w = a[:, 0, :Dd] * a[:, 0, Dd:]
    e = ((x_i * x_j).reshape(H, -1, Dd) * w[:, None, :]).sum(-1).reshape(-1)
    z = np.exp(e).astype(np.float32)
    nseg = num_nodes * H
    seg = np.zeros(nseg, np.float32)
    np.add.at(seg, idx, z)
    return (z / (seg[idx] + 1e-16)).reshape(-1, 1).astype(np.float32)


def _l2_params(counts, nseg, seg_pc):
    """Two count-classes: A = segments with count <= padA (bulk, small
    pad), B = the rare heavy tail. Returns per-class shapes plus the
    per-segment class flag and within-(core,class) position."""
    pad = int(max(4, -(-int(counts.max()) // 4) * 4))
    padA = min(16, pad)
    clsB = counts > padA
    pos = np.empty(nseg, np.int64)
    nA = np.zeros(NCORES, np.int64)
    nB = np.zeros(NCORES, np.int64)
    for c in range(NCORES):
        lo, hi = c * seg_pc, min((c + 1) * seg_pc, nseg)
        m = clsB[lo:hi]
        sub = pos[lo:hi]
        sub[~m] = np.arange(int((~m).sum()), dtype=np.int64)
        sub[m] = np.arange(int(m.sum()), dtype=np.int64)
        nA[c] = int((~m).sum())
        nB[c] = int(m.sum())
    ntA = max(1, -(-int(nA.max()) // P))
    ntB = max(1, -(-int(nB.max()) // P))
    padB = pad if clsB.any() else padA
    return ntA, padA, ntB, padB, clsB, pos


def kernel(x_i, x_j, a, edge_index, num_nodes):
    x_i = np.asarray(x_i, dtype=np.float32)
    x_j = np.asarray(x_j, dtype=np.float32)
    a = np.asarray(a, dtype=np.float32)
    idx = np.asarray(edge_index)[1].astype(np.int64)
    num_nodes = int(num_nodes)

    M, Dd = x_i.shape
    H = a.shape[0]
    epc = M // NCORES if M % NCORES == 0 else 0
    if not (
        Dd == D
        and H == NCORES
        and epc
        and epc % (P * RPP) == 0
        and (epc // (P * RPP)) % 2 == 1
    ):
        return _kernel_numpy(x_i, x_j, a, idx, num_nodes)

    nseg = num_nodes * H
    seg_pc = -(-nseg // NCORES)

    # ------------- L1: per-edge exp scores ------------------------------
    w = a[:, 0, :D] * a[:, 0, D:]  # [H, D]
    key = ("l1", epc)
    if key not in _cache:
        _cache[key] = _build_l1(epc)
    nc1 = _cache[key]
    in_maps = [
        {
            "xiw": np.ascontiguousarray(
                (x_i[c * epc : (c + 1) * epc] * w[c]).astype(np.float16)
            ),
            "xj": np.ascontiguousarray(x_j[c * epc : (c + 1) * epc].astype(np.float16)),
        }
        for c in range(NCORES)
    ]
    res1 = _run_spmd(nc1, in_maps, list(range(NCORES)))
    nchunks = epc // (P * RPP)
    z_all = np.concatenate(
        [
            res1.results[c]["z"].reshape(P, nchunks, RPP).transpose(1, 0, 2).ravel()
            for c in range(NCORES)
        ]
    )

    # ------------- host: bucket by destination segment ------------------
    counts = np.bincount(idx, minlength=nseg)
    order = np.argsort(idx, kind="stable")
    starts = np.zeros(nseg, np.int64)
    np.cumsum(counts[:-1], out=starts[1:])
    ranks = np.empty(M, np.int64)
    ranks[order] = np.arange(M, dtype=np.int64) - starts[idx[order]]

    ntA, padA, ntB, padB, clsB, pos = _l2_params(counts, nseg, seg_pc)
    c_seg = idx // seg_pc
    eB = clsB[idx]
    mA = ~eB
    pos_e = pos[idx]
    pp = np.where(eB, pos_e // ntB, pos_e // ntA)
    tt = np.where(eB, pos_e % ntB, pos_e % ntA)

    zpA = np.zeros((NCORES, P, padA, ntA), np.float16)
    zpB = np.zeros((NCORES, P, padB, ntB), np.float16)
    zpA[c_seg[mA], pp[mA], ranks[mA], tt[mA]] = z_all[mA]
    zpB[c_seg[eB], pp[eB], ranks[eB], tt[eB]] = z_all[eB]

    # ------------- L2: segment normalize --------------------------------
    key2 = ("l2", ntA, padA, ntB, padB)
    if key2 not in _cache:
        _cache[key2] = _build_l2(ntA, padA, ntB, padB)
    nc2 = _cache[key2]
    res2 = _run_spmd(
        nc2,
        [{"zpA": zpA[c], "zpB": zpB[c]} for c in range(NCORES)],
        list(range(NCORES)),
    )
    apA = np.stack([res2.results[c]["apA"] for c in range(NCORES)])
    apB = np.stack([res2.results[c]["apB"] for c in range(NCORES)])

    alpha = np.empty(M, np.float32)
    alpha[mA] = apA[c_seg[mA], pp[mA], ranks[mA], tt[mA]].astype(np.float32)
    alpha[eB] = apB[c_seg[eB], pp[eB], ranks[eB], tt[eB]].astype(np.float32)
    return alpha.reshape(-1, 1)


# revision 6
# speedup vs baseline: 1.0061x; 1.0061x over previous
"""GAT edge-softmax (segment softmax over 400K segments) on 8 Trainium2
NeuronCores, written in raw Bass — fp16 streaming version.

Structure
---------
L1 (device, DMA-bound): the 3.2M edges are sharded contiguously across
the 8 cores; with 8 heads and E edges/head, core c gets exactly head
c's edges, so the attention vector w = a_l * a_r is a per-core
constant. The host folds w and the f32->f16 conversion into one pass
(xiw = x_i * w, xj both fp16), halving HBM traffic vs f32 — a single
SP-queue DMA stream measures 283us/core for the 102.4 MB, and this
kernel reaches 292us. Compute runs in 2-chunk "super" units: one
in-place fp16 multiply (DVE 2x mode), then a halving tree for the
64-wide window sum — first step out-of-place into a small pyramid
buffer (frees the input slot early for prefetch), remaining steps in
place, all fp16 2x — then ACT Exp writes fp16 z. The per-sweep z
write-back is issued from the ACT queue, where it orders naturally
after the last Exp instead of stalling the SP DMA stream.

Host (pure index shuffling): z is bucketed by destination segment into
a dense zero-padded pad-major [pad, segments] fp16 layout,
pre-partitioned so each segment lives on exactly one core — the
cross-device segment reduction of the hint becomes unnecessary, and
the empty padding slots are exact zeros under sum.

L2 (device, small): whole-buffer single DMAs (in on SP, out on ACT
queue); DVE sums the pad axis with a halving tree (fp16 2x), adds
1e-16, reciprocal, and one 2x broadcast multiply normalizes in place.

Host: alphas are gathered back to the original edge order (f32 out).

The reference's max-subtraction is skipped: e = sum_d xi*xj*w has
sigma ~0.12 (w is glorot-initialized), so |e| < ~1 over 3.2M samples;
exp cannot overflow fp16 and alpha = z/(sum z + 1e-16) differs from
the max-subtracted form by <=2e-16 relative.

Accuracy budget: fp16 inputs + fp16 tree rounding give max rel err
~2e-3 on alpha, vs the 2e-2 gate.

Platform constraints honored (found the hard way):
- walrus permits at most ONE semaphore wait attached per instruction ->
  standalone wait instructions, no TileContext.
- dependent same-engine ops still need semaphore sync (engine frees
  before writes drain); the race detector enforces this.
- multi-queue BULK DMA is ~1.6x WORSE on real HW than a single queue
  (CoreSim models it as 2x better — do not trust it there); only the
  small per-sweep write-backs go on the ACT queue.
"""
import contextlib
import sys

sys.path.insert(0, "/opt/trn_rl_repo")

import numpy as np

import concourse.bass as bass
from concourse import mybir
from concourse.bass_utils import run_bass_kernel_spmd

F16 = mybir.dt.float16
F32 = mybir.dt.float32
P = 128
D = 64
NCORES = 8
RPP = 125  # edge rows per partition per L1 chunk

_cache = {}


def _build_l1(epc, rpp=RPP, repeat=1):
    """Per-core score kernel: z[p, c*rpp+r] = exp(sum_d xiw*xj) of edge
    c*(128*rpp) + p*rpp + r. Inputs xiw/xj [epc, 64] f16; z [128, epc/128]
    f16. Compute in 2-chunk super units; 25 chunks/sweep = 12 supers +
    tail chunk (dedicated slot 4; super chunks cycle slots 0-3)."""
    chunk_edges = P * rpp
    assert epc % chunk_edges == 0
    nchunks_data = epc // chunk_edges
    assert nchunks_data % 2 == 1
    nsup = nchunks_data // 2
    free = rpp * D
    srpp = 2 * rpp
    zcols = epc // P
    Exp = mybir.ActivationFunctionType.Exp

    nc = bass.Bass()
    xiw = nc.declare_dram_parameter("xiw", [epc, D], F16, isOutput=False)
    xj = nc.declare_dram_parameter("xj", [epc, D], F16, isOutput=False)
    z_out = nc.declare_dram_parameter("z", [P, zcols], F16, isOutput=True)

    xi_t = xiw[:].rearrange("(c p r) d -> c p (r d)", p=P, r=rpp)
    xj_t = xj[:].rearrange("(c p r) d -> c p (r d)", p=P, r=rpp)

    UPS = nsup + 1  # units per sweep: supers then the tail chunk
    nunits = UPS * repeat
    nchunks = nchunks_data * repeat

    def chunk_slot(c):
        dc = c % nchunks_data
        return 4 if dc == nchunks_data - 1 else dc % 4

    def chunk_unit(c):
        sweep, dc = divmod(c, nchunks_data)
        return sweep * UPS + min(dc // 2, nsup)

    def unit_chunks(g):
        sweep, u = divmod(g, UPS)
        base = sweep * nchunks_data
        if u < nsup:
            return [base + 2 * u, base + 2 * u + 1]
        return [base + 2 * nsup]

    slot_uses = {}
    use_idx = {}
    for c in range(nchunks):
        b = chunk_slot(c)
        slot_uses[b] = slot_uses.get(b, 0) + 1
        use_idx[c] = slot_uses[b]

    # DVE ops per unit: mult, t1 (out-of-place), t2..t5 (in-place), t6.
    # Units run sequentially except the sweep's LAST two (super 11 +
    # tail chunk), which are zipped op-by-op: their DMAs are already
    # prefetched by then, so the zip hides the write-drain latency of
    # the tail's 7 small serial ops at the sweep boundary (measured
    # -12us/sweep). Zipping ALL pairs regresses (DMA gating stalls).
    order = []
    for sweep in range(repeat):
        base = sweep * UPS
        for u in range(UPS - 2):
            order.extend((base + u, k) for k in range(7))
        for k in range(7):
            order.append((base + UPS - 2, k))
            order.append((base + UPS - 1, k))
    val = {}
    n = 0
    for g, k in order:
        n += 1
        val[(g, k)] = n

    st = contextlib.ExitStack()
    with st:
        ti = st.enter_context(nc.sbuf_tensor("ti", [P, 5 * free], F16))
        tj = st.enter_context(nc.sbuf_tensor("tj", [P, 5 * free], F16))
        u1 = [st.enter_context(nc.sbuf_tensor(f"u1{k}", [P, srpp * 32], F16)) for k in range(2)]
        er = [st.enter_context(nc.sbuf_tensor(f"er{k}", [P, srpp], F16)) for k in range(2)]
        zbuf = st.enter_context(nc.sbuf_tensor("zbuf", [P, zcols], F16))
        smi = [st.enter_context(nc.semaphore(f"smi{k}")) for k in range(5)]
        smj = [st.enter_context(nc.semaphore(f"smj{k}")) for k in range(5)]
        dve_sem = st.enter_context(nc.semaphore("dve_sem"))
        act_sem = st.enter_context(nc.semaphore("act_sem"))
        out_sem = st.enter_context(nc.semaphore("out_sem"))
        block = st.enter_context(nc.Block())

        @block.sync
        def _(sync):
            prev_use = {}
            for c in range(nchunks):
                b = chunk_slot(c)
                if b in prev_use:
                    # slot reuse: the unit that consumed the previous
                    # occupant must have finished t1 (frees ti+tj)
                    sync.wait_ge(dve_sem, val[(chunk_unit(prev_use[b]), 1)])
                prev_use[b] = c
                dc = c % nchunks_data
                sync.dma_start(
                    out=ti[:, b * free : (b + 1) * free], in_=xi_t[dc]
                ).then_inc(smi[b], 16)
                sync.dma_start(
                    out=tj[:, b * free : (b + 1) * free], in_=xj_t[dc]
                ).then_inc(smj[b], 16)
            sync.wait_ge(out_sem, 16 * repeat)

        @block.vector
        def _(vector):
            for g, k in order:
                chunks = unit_chunks(g)
                b0 = chunk_slot(chunks[0])
                width = srpp if len(chunks) == 2 else rpp
                tiv = ti[:, b0 * free : b0 * free + width * D]
                tjv = tj[:, b0 * free : b0 * free + width * D]
                ub = u1[g % 2]
                eb = er[g % 2]
                uv = ub[:, : width * 32].rearrange("p (r w) -> p r w", w=32)
                if k == 0:
                    for c in chunks:
                        vector.wait_ge(smi[chunk_slot(c)], 16 * use_idx[c])
                        vector.wait_ge(smj[chunk_slot(c)], 16 * use_idx[c])
                    nc.vector.tensor_tensor(
                        out=tiv, in0=tiv, in1=tjv, op=mybir.AluOpType.mult
                    ).then_inc(dve_sem, 1)
                elif k == 1:
                    if g >= 2:
                        # u1[g%2] reuse: unit g-2's t6 must have read it
                        vector.wait_ge(dve_sem, val[(g - 2, 6)])
                    vector.wait_ge(dve_sem, val[(g, 0)])
                    tv = tiv.rearrange("p (r d) -> p r d", d=D)
                    nc.vector.tensor_tensor(
                        out=uv, in0=tv[:, :, 0:32], in1=tv[:, :, 32:64],
                        op=mybir.AluOpType.add,
                    ).then_inc(dve_sem, 1)
                elif k < 6:
                    w = 32 >> (k - 1)  # 16, 8, 4, 2
                    vector.wait_ge(dve_sem, val[(g, k - 1)])
                    nc.vector.tensor_tensor(
                        out=uv[:, :, 0:w], in0=uv[:, :, 0:w],
                        in1=uv[:, :, w : 2 * w], op=mybir.AluOpType.add,
                    ).then_inc(dve_sem, 1)
                else:
                    if g >= 2:
                        # er[g%2] reuse: ACT of unit g-2 must have read it
                        vector.wait_ge(act_sem, g - 1)
                    vector.wait_ge(dve_sem, val[(g, 5)])
                    nc.vector.tensor_tensor(
                        out=eb[:, :width].rearrange("p (r o) -> p r o", o=1),
                        in0=uv[:, :, 0:1], in1=uv[:, :, 1:2],
                        op=mybir.AluOpType.add,
                    ).then_inc(dve_sem, 1)

        @block.scalar
        def _(scalar):
            for g in range(nunits):
                sweep, u = divmod(g, UPS)
                chunks = unit_chunks(g)
                width = srpp if len(chunks) == 2 else rpp
                col0 = (chunks[0] % nchunks_data) * rpp
                if u == 0 and sweep >= 1:
                    # zbuf overwrite must not race the async z_out read
                    scalar.wait_ge(out_sem, 16 * sweep)
                scalar.wait_ge(dve_sem, val[(g, 6)])
                nc.scalar.activation(
                    out=zbuf[:, col0 : col0 + width],
                    in_=er[g % 2][:, :width],
                    func=Exp,
                ).then_inc(act_sem, 1)
                if u == UPS - 1:
                    # sweep's last exp drained -> write z back; in-order
                    # ACT queue also orders this before next sweep's exps
                    scalar.wait_ge(act_sem, UPS * (sweep + 1))
                    if sweep >= 1:
                        scalar.wait_ge(out_sem, 16 * sweep)
                    nc.scalar.dma_start(out=z_out[:], in_=zbuf[:]).then_inc(
                        out_sem, 16
                    )

    return nc


def _tree_steps(pad):
    steps = []
    q = pad
    while q > 2:
        h = q // 2
        steps.append((h, q))
        q = q - h
    return steps


def _build_l2(ntA, padA, ntB, padB, repeat=1):
    """Per-core segment normalize, two count-classes, pad-major fp16:
    ap[p,q,t] = zp[p,q,t] / (sum_q zp[p,q,t] + 1e-16) for each class.
    Class B (ntB=0 disallowed; pass ntB>=1 zero-filled when empty)."""
    assert padA % 2 == 0 and padA >= 4 and padB % 2 == 0 and padB >= 4
    nc = bass.Bass()
    zpA = nc.declare_dram_parameter("zpA", [P, padA, ntA], F16, isOutput=False)
    zpB = nc.declare_dram_parameter("zpB", [P, padB, ntB], F16, isOutput=False)
    apA = nc.declare_dram_parameter("apA", [P, padA, ntA], F16, isOutput=True)
    apB = nc.declare_dram_parameter("apB", [P, padB, ntB], F16, isOutput=True)

    phases = [
        dict(nt=ntA, pad=padA, steps=_tree_steps(padA)),
        dict(nt=ntB, pad=padB, steps=_tree_steps(padB)),
    ]
    for ph in phases:
        ph["dops"] = len(ph["steps"]) + 5
    DOPS = sum(ph["dops"] for ph in phases)
    w1_elems = max((ph["pad"] // 2) * ph["nt"] for ph in phases)
    s_elems = max(ph["nt"] for ph in phases)

    st = contextlib.ExitStack()
    with st:
        zbA = [st.enter_context(nc.sbuf_tensor(f"zbA{k}", [P, padA * ntA], F16)) for k in range(2)]
        zbB = [st.enter_context(nc.sbuf_tensor(f"zbB{k}", [P, padB * ntB], F16)) for k in range(2)]
        w1 = st.enter_context(nc.sbuf_tensor("w1", [P, w1_elems], F16))
        s = st.enter_context(nc.sbuf_tensor("s", [P, s_elems], F32))
        rec = st.enter_context(nc.sbuf_tensor("rec", [P, s_elems], F16))
        sminA = [st.enter_context(nc.semaphore(f"sminA{k}")) for k in range(2)]
        sminB = [st.enter_context(nc.semaphore(f"sminB{k}")) for k in range(2)]
        dve_sem = st.enter_context(nc.semaphore("dve_sem"))
        outA_sem = st.enter_context(nc.semaphore("outA_sem"))
        outB_sem = st.enter_context(nc.semaphore("outB_sem"))
        block = st.enter_context(nc.Block())

        phases[0].update(zb=zbA, smin=sminA, out_sem=outA_sem, zp=zpA, ap=apA)
        phases[1].update(zb=zbB, smin=sminB, out_sem=outB_sem, zp=zpB, ap=apB)

        @block.sync
        def _(sync):
            for sw in range(repeat):
                b = sw % 2
                for ph in phases:
                    if sw >= 2:
                        sync.wait_ge(ph["out_sem"], 16 * (sw - 1))
                    sync.dma_start(out=ph["zb"][b][:], in_=ph["zp"][:]).then_inc(
                        ph["smin"][b], 16
                    )
            for ph in phases:
                sync.wait_ge(ph["out_sem"], 16 * repeat)

        @block.vector
        def _(vector):
            for sw in range(repeat):
                b = sw % 2
                k = DOPS * sw  # running dve_sem value
                for pi, ph in enumerate(phases):
                    nt, pad = ph["nt"], ph["pad"]
                    vector.wait_ge(ph["smin"][b], 16 * (sw // 2 + 1))
                    if sw >= 1 and pi == 0:
                        # w1/s/rec write-after-read vs prev sweep's phase B
                        vector.wait_ge(dve_sem, DOPS * sw)
                    zv = ph["zb"][b][:].rearrange("p (q t) -> p q t", t=nt)
                    wv = w1[:, : (pad // 2) * nt].rearrange(
                        "p (q t) -> p q t", t=nt
                    )
                    first = True
                    for h, qq in ph["steps"]:
                        if first:
                            if pi == 1:
                                # w1 write-after-read vs phase A's final
                                vector.wait_ge(dve_sem, k)
                            nc.vector.tensor_tensor(
                                out=wv[:, 0:h, :], in0=zv[:, 0:h, :],
                                in1=zv[:, qq - h : qq, :],
                                op=mybir.AluOpType.add,
                            ).then_inc(dve_sem, 1)
                        else:
                            vector.wait_ge(dve_sem, k)
                            nc.vector.tensor_tensor(
                                out=wv[:, 0:h, :], in0=wv[:, 0:h, :],
                                in1=wv[:, qq - h : qq, :],
                                op=mybir.AluOpType.add,
                            ).then_inc(dve_sem, 1)
                        first = False
                        k += 1
                    vector.wait_ge(dve_sem, k)
                    nc.vector.tensor_tensor(
                        out=s[:, :nt].rearrange("p (o t) -> p o t", o=1),
                        in0=wv[:, 0:1, :], in1=wv[:, 1:2, :],
                        op=mybir.AluOpType.add,
                    ).then_inc(dve_sem, 1)
                    k += 1
                    vector.wait_ge(dve_sem, k)
                    nc.vector.tensor_scalar_add(
                        out=s[:, :nt], in0=s[:, :nt], scalar1=1e-16
                    ).then_inc(dve_sem, 1)
                    k += 1
                    vector.wait_ge(dve_sem, k)
                    nc.vector.reciprocal(out=s[:, :nt], in_=s[:, :nt]).then_inc(
                        dve_sem, 1
                    )
                    k += 1
                    vector.wait_ge(dve_sem, k)
                    # clamped f16 cast: empty segments have recip 1e16
                    # which would overflow f16; real segments are < 3
                    nc.vector.tensor_scalar(
                        out=rec[:, :nt], in0=s[:, :nt], scalar1=60000.0,
                        scalar2=None, op0=mybir.AluOpType.min,
                    ).then_inc(dve_sem, 1)
                    k += 1
                    vector.wait_ge(dve_sem, k)
                    rec_ap = rec[:, :nt]
                    rb = bass.AP(
                        tensor=rec_ap.tensor, offset=rec_ap.offset,
                        ap=[rec_ap.ap[0], [0, pad], rec_ap.ap[1]],
                    )
                    nc.vector.tensor_tensor(
                        out=zv, in0=zv, in1=rb, op=mybir.AluOpType.mult
                    ).then_inc(dve_sem, 1)
                    k += 1

        @block.scalar
        def _(scalar):
            for sw in range(repeat):
                b = sw % 2
                k = DOPS * sw
                for ph in phases:
                    k += ph["dops"]
                    scalar.wait_ge(dve_sem, k)
                    if sw >= 1:
                        scalar.wait_ge(ph["out_sem"], 16 * sw)
                    nc.scalar.dma_start(
                        out=ph["ap"][:], in_=ph["zb"][b][:]
                    ).then_inc(ph["out_sem"], 16)

    return nc


def _run_spmd(nc, in_maps, core_ids, tries=3):
    last = None
    for attempt in range(tries):
        try:
            return run_bass_kernel_spmd(nc, in_maps, core_ids)
        except Exception as e:  # axon/NRT execution is occasionally flaky
            last = e
    raise last


def _kernel_numpy(x_i, x_j, a, idx, num_nodes):
    """Host fallback for shapes the device path doesn't cover."""
    H = a.shape[0]
    Dd = a.shape[2] // 2
    w = a[:, 0, :Dd] * a[:, 0, Dd:]
    e = ((x_i * x_j).reshape(H, -1, Dd) * w[:, None, :]).sum(-1).reshape(-1)
    z = np.exp(e).astype(np.float32)
    nseg = num_nodes * H
    seg = np.zeros(nseg, np.float32)
    np.add.at(seg, idx, z)
    return (z / (seg[idx] + 1e-16)).reshape(-1, 1).astype(np.float32)


def _l2_params(counts, nseg, seg_pc):
    """Two count-classes: A = segments with count <= padA (bulk, small
    pad), B = the rare heavy tail. Returns per-class shapes plus the
    per-segment class flag and within-(core,class) position."""
    pad = int(max(4, -(-int(counts.max()) // 4) * 4))
    padA = min(16, pad)
    clsB = counts > padA
    pos = np.empty(nseg, np.int64)
    nA = np.zeros(NCORES, np.int64)
    nB = np.zeros(NCORES, np.int64)
    for c in range(NCORES):
        lo, hi = c * seg_pc, min((c + 1) * seg_pc, nseg)
        m = clsB[lo:hi]
        sub = pos[lo:hi]
        sub[~m] = np.arange(int((~m).sum()), dtype=np.int64)
        sub[m] = np.arange(int(m.sum()), dtype=np.int64)
        nA[c] = int((~m).sum())
        nB[c] = int(m.sum())
    ntA = max(1, -(-int(nA.max()) // P))
    ntB = max(1, -(-int(nB.max()) // P))
    padB = pad if clsB.any() else padA
    return ntA, padA, ntB, padB, clsB, pos


def kernel(x_i, x_j, a, edge_index, num_nodes):
    x_i = np.asarray(x_i, dtype=np.float32)
    x_j = np.asarray(x_j, dtype=np.float32)
    a = np.asarray(a, dtype=np.float32)
    idx = np.asarray(edge_index)[1].astype(np.int64)
    num_nodes = int(num_nodes)

    M, Dd = x_i.shape
    H = a.shape[0]
    epc = M // NCORES if M % NCORES == 0 else 0
    if not (
        Dd == D
        and H == NCORES
        and epc
        and epc % (P * RPP) == 0
        and (epc // (P * RPP)) % 2 == 1
    ):
        return _kernel_numpy(x_i, x_j, a, idx, num_nodes)

    nseg = num_nodes * H
    seg_pc = -(-nseg // NCORES)

    # ------------- L1: per-edge exp scores ------------------------------
    w = a[:, 0, :D] * a[:, 0, D:]  # [H, D]
    key = ("l1", epc)
    if key not in _cache:
        _cache[key] = _build_l1(epc)
    nc1 = _cache[key]
    in_maps = [
        {
            "xiw": np.ascontiguousarray(
                (x_i[c * epc : (c + 1) * epc] * w[c]).astype(np.float16)
            ),
            "xj": np.ascontiguousarray(x_j[c * epc : (c + 1) * epc].astype(np.float16)),
        }
        for c in range(NCORES)
    ]
    res1 = _run_spmd(nc1, in_maps, list(range(NCORES)))
    nchunks = epc // (P * RPP)
    z_all = np.concatenate(
        [
            res1.results[c]["z"].reshape(P, nchunks, RPP).transpose(1, 0, 2).ravel()
            for c in range(NCORES)
        ]
    )

    # ------------- host: bucket by destination segment ------------------
    counts = np.bincount(idx, minlength=nseg)
    order = np.argsort(idx, kind="stable")
    starts = np.zeros(nseg, np.int64)
    np.cumsum(counts[:-1], out=starts[1:])
    ranks = np.empty(M, np.int64)
    ranks[order] = np.arange(M, dtype=np.int64) - starts[idx[order]]

    ntA, padA, ntB, padB, clsB, pos = _l2_params(counts, nseg, seg_pc)
    c_seg = idx // seg_pc
    eB = clsB[idx]
    mA = ~eB
    pos_e = pos[idx]
    pp = np.where(eB, pos_e // ntB, pos_e // ntA)
    tt = np.where(eB, pos_e % ntB, pos_e % ntA)

    zpA = np.zeros((NCORES, P, padA, ntA), np.float16)
    zpB = np.zeros((NCORES, P, padB, ntB), np.float16)
    zpA[c_seg[mA], pp[mA], ranks[mA], tt[mA]] = z_all[mA]
    zpB[c_seg[eB], pp[eB], ranks[eB], tt[eB]] = z_all[eB]

    # ------------- L2: segment normalize --------------------------------
    key2 = ("l2", ntA, padA, ntB, padB)
    if key2 not in _cache:
        _cache[key2] = _build_l2(ntA, padA, ntB, padB)
    nc2 = _cache[key2]
    res2 = _run_spmd(
        nc2,
        [{"zpA": zpA[c], "zpB": zpB[c]} for c in range(NCORES)],
        list(range(NCORES)),
    )
    apA = np.stack([res2.results[c]["apA"] for c in range(NCORES)])
    apB = np.stack([res2.results[c]["apB"] for c in range(NCORES)])

    alpha = np.empty(M, np.float32)
    alpha[mA] = apA[c_seg[mA], pp[mA], ranks[mA], tt[mA]].astype(np.float32)
    alpha[eB] = apB[c_seg[eB], pp[eB], ranks[eB], tt[eB]].astype(np.float32)
    return alpha.reshape(-1, 1)
